# revision 1
# baseline (speedup 1.0000x reference)
"""Trainium2 Bass kernel for nn_LocalAtten (local attention block).

Reference computation (per sample):
  xr    = relu(conv1x1(x; w1, b1))                  # (CI=16, H, W)
  attn  = softmax(relu(conv1x1(x; w2, b2)), axis=k) # (9, H, W)
  S     = sum_k attn[k] * shift(xr, k)              # 3x3 window, zero pad
  out   = x + relu(conv1x1(S; w3, b3))              # (C=256, H, W)

Sharding: data-parallel over N; core i processes samples [2i, 2i+1].

Layout strategy (per core):
  - x is loaded channel-major: (128 c-chunk partitions, 32 h, 128 w) quarter
    tiles; these double as the residual / output staging (in-place add).
  - conv1+conv2 are fused: one matmul per image row with the x row-block as
    the STATIONARY operand (lhsT = x[c, w], rhs = W12T[c, 25]) so the output
    (w partitions, 25) is pixel-major. Biases b1/b2 are pre-filled into the
    PSUM bank by a leading K=1 ones-matmul with start=True.
  - softmax over the 9 logits is done pixel-major.  exp(relu(z)) == max(exp(z), 1)
    so the branch relu folds into a tensor_scalar_max.
  - w-shifted copies of attn (partition shifts) are made with PE matmuls
    against shifted identities (engines cannot access partition base != 0 mod 32).
  - the 3x3 stencil is computed as three tensors A_dj (dj = w-offset), each
    using only free-dim h-shifts on DVE in bf16 with attn broadcast via
    step-0 access patterns:  A_dj[w] = sum_di attn_k(di,dj)[w-dj] * xr[w, h+di]
  - channel-major S is then built by PSUM-accumulating matmuls
    pt[j, n] += sum_w A_dj[w, j] * I_dj[w, n], with I_dj = identity shifted by
    dj -- this performs transpose AND w-shift in one op, with automatic zero
    boundaries.  Block layout: j = 16*h_sub + c over 8-row blocks.
  - conv3: K=128 matmuls against host-built h_sub-selective block weight
    matrices (zero rows kill the 7 other rows packed in the block) so every
    operand stays at partition base 0 (nonzero-base matmuls fault on HW);
    relu+b3 on ACT (per-partition bias); residual add on DVE in-place into
    the x tiles, which are DMA'd out.
  - all tail phases run per 32-row quarter so they pipeline against later
    conv12 banks, outputs stream out early, and the freed x slots let the
    next sample's loads keep the DMA engines busy.
"""

import numpy as np
import ml_dtypes

import concourse.bass as bass
import concourse.bacc as bacc
import concourse.tile as tile
from concourse import mybir
from concourse.bass_utils import run_bass_kernel_spmd

F32 = mybir.dt.float32
BF16 = mybir.dt.bfloat16
AFT = mybir.ActivationFunctionType
AX = mybir.AxisListType

N_CORES = 8
NS = 2            # samples per core
C = 256
CI = 16
NK = 9
H = 128
W = 128
REG = 28          # psum col stride per row region in conv12 bank (25 used)
RPB = 16          # rows per conv12 psum bank

# packed-constant layouts
OFF_W12 = 0                     # 2 chunks x REG cols (f32)
OFF_B3 = 2 * REG                # 2 cols (f32)
OFF_B12 = OFF_B3 + 2            # RPB*REG cols, partition 0 (f32)
OFF_ONES = OFF_B12 + RPB * REG  # 128 cols of 1.0, partition 0 (f32)
CF_LEN = OFF_ONES + 128
OFF_W3S = 0                     # 8 h_sub x 2 oh x 128 cols (bf16, h_sub-
                                # selective block weights, rows j=16*hs+c)
OFF_ID = 8 * 2 * 128            # 3 x 128 identity cols (bf16)
CB_LEN = OFF_ID + 3 * 128


def _build_module():
    nc = bacc.Bacc("TRN2")
    x_d = nc.declare_dram_parameter("x", [NS, C, H, W], F32, isOutput=False)
    cf_d = nc.declare_dram_parameter("cf32", [128, CF_LEN], F32, isOutput=False)
    cb_d = nc.declare_dram_parameter("cbf16", [128, CB_LEN], BF16,
                                     isOutput=False)
    y_d = nc.declare_dram_parameter("y", [NS, C, H, W], F32, isOutput=True)

    from contextlib import ExitStack
    with tile.TileContext(nc) as tc, ExitStack() as ctx:
        consts = ctx.enter_context(tc.tile_pool(name="consts", bufs=1))
        xq_pool = ctx.enter_context(tc.tile_pool(name="xq", bufs=9))
        xr_pool = ctx.enter_context(tc.tile_pool(name="xr", bufs=2))
        att_pool = ctx.enter_context(tc.tile_pool(name="att", bufs=2))
        sm_pool = ctx.enter_context(tc.tile_pool(name="sm", bufs=1))
        a_pool = ctx.enter_context(tc.tile_pool(name="apool", bufs=1))
        a2_pool = ctx.enter_context(tc.tile_pool(name="a2pool", bufs=2))
        tmp_pool = ctx.enter_context(tc.tile_pool(name="tmp", bufs=1))
        scm_pool = ctx.enter_context(tc.tile_pool(name="scm", bufs=1))
        t_pool = ctx.enter_context(tc.tile_pool(name="tst", bufs=2))
        pc12 = ctx.enter_context(tc.tile_pool(name="pc12", bufs=2, space="PSUM"))
        pT = ctx.enter_context(tc.tile_pool(name="pT", bufs=2, space="PSUM"))
        pwarm = ctx.enter_context(tc.tile_pool(name="pwarm", bufs=1,
                                               space="PSUM"))
        p3 = ctx.enter_context(tc.tile_pool(name="p3", bufs=2, space="PSUM"))

        # ---- constants: two packed tiles -> one DMA sem each ----
        cf = consts.tile([128, CF_LEN], F32)
        nc.sync.dma_start(out=cf[:], in_=cf_d[:])
        cb = consts.tile([128, CB_LEN], BF16)
        nc.sync.dma_start(out=cb[:], in_=cb_d[:])
        # tiny warm-up matmuls absorb the const-DMA waits on the PE queue so
        # no later matmul carries two sync waits (LDWEIGHTS wait-slot limit)
        warm = pwarm.tile([1, 2], F32, tag="warm")
        nc.tensor.matmul(out=warm[0:1, 0:1], lhsT=cf[0:1, 0:1],
                         rhs=cf[0:1, 0:1], start=True, stop=True)
        nc.tensor.matmul(out=warm[0:1, 1:2], lhsT=cb[0:1, 0:1],
                         rhs=cb[0:1, 0:1], start=True, stop=True)

        for s in range(NS):
            # ---- load x: 2 c-chunks x 4 h-quarters ----
            xq = {}
            for cc in range(2):
                for q in range(4):
                    t = xq_pool.tile([128, 32, W], F32, tag="xq")
                    nc.sync.dma_start(
                        out=t[:],
                        in_=x_d[s, cc * 128:(cc + 1) * 128, 32 * q:32 * (q + 1), :],
                    )
                    xq[(cc, q)] = t

            # pixel-major intermediates: partition = w
            xr = xr_pool.tile([128, CI, H + 2], BF16)       # (w, c, hpad)
            att = att_pool.tile([128, NK, H], BF16)         # (w, k, h)
            nc.vector.memset(xr[:, :, 0:1], 0.0)
            nc.vector.memset(xr[:, :, H + 1:H + 2], 0.0)

            # ---- conv1+conv2 fused, 8 banks of 16 rows ----
            for b in range(H // RPB):
                ps = pc12.tile([128, RPB, REG], F32, tag="ps")
                # bias pre-fill: clears has_written for the bank, writes b12
                # into every row region (start=True)
                nc.tensor.matmul(
                    out=ps[:].rearrange("p a b -> p (a b)"),
                    lhsT=cf[0:1, OFF_ONES:OFF_ONES + 128],
                    rhs=cf[0:1, OFF_B12:OFF_B12 + RPB * REG],
                    start=True, stop=False,
                )
                for r in range(RPB):
                    h = RPB * b + r
                    q, hl = divmod(h, 32)
                    for cc in range(2):
                        nc.tensor.matmul(
                            out=ps[:, r, 0:CI + NK],
                            lhsT=xq[(cc, q)][:, hl, :],
                            rhs=cf[:, cc * REG:cc * REG + CI + NK],
                            start=False,
                            stop=(r == RPB - 1 and cc == 1),
                        )
                # xr rows (relu): psum (128, 16r, 16c) -> xr (w, c, 1+h)
                nc.scalar.activation(
                    out=xr[:, :, 1 + RPB * b:1 + RPB * (b + 1)].transpose([0, 2, 1]),
                    in_=ps[:, :, 0:CI],
                    func=AFT.Relu,
                )
                # attention logits -> exp (relu folded in later via max(,1))
                nc.scalar.activation(
                    out=att[:, :, RPB * b:RPB * (b + 1)].transpose([0, 2, 1]),
                    in_=ps[:, :, CI:CI + NK],
                    func=AFT.Exp,
                )

            # tail phases processed per h-half so they overlap later conv12
            # banks and release x tiles (and start output DMAs) early
            att_m1 = att_pool.tile([128, 3, H], BF16, tag="attm1")
            att_p1 = att_pool.tile([128, 3, H], BF16, tag="attp1")
            sums = sm_pool.tile([128, H], F32, tag="sums")
            recip = sm_pool.tile([128, H], F32, tag="recip")
            A = a_pool.tile([128, 3, CI, H], BF16)          # (w, dj, c, h)
            tmp = tmp_pool.tile([128, CI, H], BF16)
            scm = scm_pool.tile([128, 16, 128], BF16)
            groups = (
                (0, att, (1, 4, 7)),      # dj = 0
                (1, att_m1, (0, 1, 2)),   # dj = -1 (ks 0,3,6 pre-gathered)
                (2, att_p1, (0, 1, 2)),   # dj = +1 (ks 2,5,8 pre-gathered)
            )
            for g4 in range(4):
                h0 = 32 * g4
                HL = 32

                # ---- softmax over k (pixel-major) ----
                attv = att[:, :, h0:h0 + HL]
                nc.vector.tensor_scalar_max(out=attv, in0=attv, scalar1=1.0)
                nc.vector.reduce_sum(out=sums[:, h0:h0 + HL],
                                     in_=attv.transpose([0, 2, 1]), axis=AX.X)
                nc.vector.reciprocal(out=recip[:, h0:h0 + HL],
                                     in_=sums[:, h0:h0 + HL])
                nc.vector.tensor_mul(
                    out=attv, in0=attv,
                    in1=recip[:, h0:h0 + HL].unsqueeze(1)
                        .broadcast_to((128, NK, HL)),
                )

                # ---- w-shifted attn copies via PE (shifted identity) ----
                # att_m1[w] = att[w+1] (dj=-1 taps k=0,3,6)
                # att_p1[w] = att[w-1] (dj=+1 taps k=2,5,8)
                for (dst, ident_i, k0) in ((att_m1, 2, 0), (att_p1, 1, 2)):
                    psh = pT.tile([128, 3, HL], F32, tag="pt")
                    for kk in range(3):
                        nc.tensor.matmul(
                            out=psh[:, kk, :],
                            lhsT=cb[:, OFF_ID + ident_i * 128:
                                    OFF_ID + (ident_i + 1) * 128],
                            rhs=att[:, k0 + 3 * kk, h0:h0 + HL],
                            start=True, stop=True,
                        )
                    nc.scalar.copy(out=dst[:, :, h0:h0 + HL], in_=psh[:])

                # ---- 3x3 stencil -> A_dj tensors (free-dim shifts only) ----
                for idx, asrc, ks in groups:
                    for j_i, di in enumerate((-1, 0, 1)):
                        kk = ks[j_i]
                        out_v = A[:, idx, :, h0:h0 + HL]
                        in0_v = xr[:, :, 1 + h0 + di:1 + h0 + di + HL]
                        in1_v = asrc[:, kk:kk + 1, h0:h0 + HL] \
                            .broadcast_to((128, CI, HL))
                        if j_i == 0:
                            nc.vector.tensor_mul(out=out_v, in0=in0_v, in1=in1_v)
                        else:
                            tmp_v = tmp[:, :, h0:h0 + HL]
                            nc.vector.tensor_mul(out=tmp_v, in0=in0_v, in1=in1_v)
                            nc.vector.tensor_add(out=out_v, in0=out_v, in1=tmp_v)

                # ---- transpose + w-shift + dj-sum into channel-major ----
                # Per 32-row group: reformat A (dj, c, h) -> A2 (dj, h_l, c)
                # on ACT, then pt[j, n] += sum_w A2_dj[w, j] * I_dj[w, n] with
                # j = 16*h_sub + c over 8-row blocks (partition base 0).
                A2 = a2_pool.tile([128, 3, 32, CI], BF16, tag="a2")
                for idx in range(3):
                    in_v = bass.AP(
                        tensor=A[:].tensor,
                        offset=A[:].offset + idx * (CI * H) + 32 * g4,
                        ap=[A[:].ap[0], [1, 32], [H, CI]],
                    )
                    nc.scalar.copy(out=A2[:, idx, :, :], in_=in_v)
                for bl in range(4):
                    blk = 4 * g4 + bl            # 8-row block index
                    pt = pT.tile([128, 128], F32, tag="pt")
                    for t_i, (idx, ident_i) in enumerate(
                            ((0, 0), (1, 1), (2, 2))):
                        lhs_v = A2[:, idx, 8 * bl:8 * (bl + 1), :] \
                            .rearrange("p a b -> p (a b)")
                        nc.tensor.matmul(
                            out=pt[:],
                            lhsT=lhs_v,
                            rhs=cb[:, OFF_ID + ident_i * 128:
                                   OFF_ID + (ident_i + 1) * 128],
                            start=(t_i == 0), stop=(t_i == 2),
                        )
                    nc.scalar.copy(out=scm[:, blk, :], in_=pt[:])

                # ---- conv3 + relu(+b3) + residual add + store ----
                # K=128 matmuls with h_sub-selective block weights (zeros
                # kill the other 7 rows in the same scm block) -- base 0.
                for q in (g4,):
                    for oh in range(2):
                        for gl in range(8):
                            blk = 8 * q + gl      # 4 h-rows: h = 4*blk + rr
                            pp = p3.tile([128, 4, 128], F32, tag="pp")
                            for rr in range(4):
                                h = 4 * blk + rr
                                hs = h % 8
                                nc.tensor.matmul(
                                    out=pp[:, rr, :],
                                    lhsT=cb[:, OFF_W3S + hs * 256 + oh * 128:
                                            OFF_W3S + hs * 256 + (oh + 1) * 128],
                                    rhs=scm[:, h // 8, :],
                                    start=True, stop=True,
                                )
                            tt = t_pool.tile([128, 4, 128], F32, tag="tt")
                            nc.scalar.activation(
                                out=tt[:], in_=pp[:], func=AFT.Relu,
                                bias=cf[:, OFF_B3 + oh:OFF_B3 + oh + 1],
                                scale=1.0,
                            )
                            hl = (4 * blk) % 32
                            xv = xq[(oh, q)][:, hl:hl + 4, :]
                            nc.vector.tensor_add(out=xv, in0=tt[:], in1=xv)
                        nc.sync.dma_start(
                            out=y_d[s, oh * 128:(oh + 1) * 128,
                                    32 * q:32 * (q + 1), :],
                            in_=xq[(oh, q)][:],
                        )
    nc.compile()
    return nc


_NC_CACHE = None


def _get_nc():
    global _NC_CACHE
    if _NC_CACHE is None:
        _NC_CACHE = _build_module()
    return _NC_CACHE


def _make_const_inputs(w1, b1, w2, b2, w3, b3):
    cf = np.zeros((128, CF_LEN), np.float32)
    for cc in range(2):
        cf[:, cc * REG:cc * REG + CI] = w1[:, cc * 128:(cc + 1) * 128].T
        cf[:, cc * REG + CI:cc * REG + CI + NK] = \
            w2[:, cc * 128:(cc + 1) * 128].T
    cf[:, OFF_B3:OFF_B3 + 2] = np.ascontiguousarray(b3.reshape(2, 128).T)
    b12 = np.concatenate([b1, b2]).astype(np.float32)
    for r in range(RPB):
        cf[0, OFF_B12 + r * REG:OFF_B12 + r * REG + CI + NK] = b12
    cf[0, OFF_ONES:OFF_ONES + 128] = 1.0

    cb = np.zeros((128, CB_LEN), np.float32)
    for hs in range(8):
        for oh in range(2):
            col = OFF_W3S + hs * 256 + oh * 128
            cb[16 * hs:16 * hs + CI, col:col + 128] = \
                w3[oh * 128:(oh + 1) * 128, :].T
    # idents: [0] = I (dj=0), [1] = eye(k=1) (w = n-1), [2] = eye(k=-1)
    for i, mat in enumerate((np.eye(128), np.eye(128, k=1),
                             np.eye(128, k=-1))):
        cb[:, OFF_ID + i * 128:OFF_ID + (i + 1) * 128] = mat
    return {"cf32": cf, "cbf16": cb.astype(ml_dtypes.bfloat16)}


def run(x, w1, b1, w2, b2, w3, b3, trace=False):
    x = np.ascontiguousarray(np.asarray(x, dtype=np.float32))
    consts = _make_const_inputs(
        np.asarray(w1, np.float32), np.asarray(b1, np.float32),
        np.asarray(w2, np.float32), np.asarray(b2, np.float32),
        np.asarray(w3, np.float32), np.asarray(b3, np.float32))
    nc = _get_nc()
    in_maps = []
    for core in range(N_CORES):
        m = {"x": x[NS * core:NS * (core + 1)]}
        m.update(consts)
        in_maps.append(m)
    res = run_bass_kernel_spmd(nc, in_maps, list(range(N_CORES)), trace=trace)
    y = np.concatenate([res.results[i]["y"] for i in range(N_CORES)], axis=0)
    return y, res


def kernel(**inputs):
    y, _ = run(**inputs)
    return y

